# revision 1
# baseline (speedup 1.0000x reference)
"""Trainium2 Bass kernel for multi-head attention (B=4, T=2048, HID=1024, H=16, D=64).

Sharding (8 NeuronCores): core c owns batch b = c//2 and query rows
g = c%2 (1024 of 2048). Each core projects q/k/v for its own 1024 rows;
the k/v projections are exchanged inside the (2b, 2b+1) core pair with a
2-core AllGather so every core attends over the full 2048 keys of its batch.
The final output projection is row-parallel, so the per-core outputs tile the
full [4, 2048, 1024] result with no reduction.

Numerics: every matmul runs in float32r (full-rate reduced-precision fp32 PE
mode, ~1.5e-4 per-matmul relative error). Attention probabilities are computed
by the Scalar engine directly out of PSUM with exp(S/8 + bias), the -1e9
pad-mask bias folded into the per-partition bias operand. The softmax
denominator comes from a ones-column appended to the V operand of the
probability @ V matmul, and normalization happens on the small [65, q] context
output instead of the big [k, q] probability matrix. Scores for two heads with
the same bias row (h and h+4) are computed concurrently in the PE array via
row tile_position packing (contraction dim is only D=64) and share one big
[128, 2048] exp activation.
"""

from contextlib import ExitStack

import numpy as np

import concourse.bacc as bacc
import concourse.mybir as mybir
import concourse.tile as tile
from concourse.masks import make_identity

F32 = mybir.dt.float32
R32 = mybir.dt.float32r
I32 = mybir.dt.int32
EXP = mybir.ActivationFunctionType.Exp

B, T, HID, H, D = 4, 2048, 1024, 16, 64
TL = T // 2           # query rows owned by one core
N_CORES = 8
NEG_INF = -1.0e9
SCALE = float(D) ** -0.5

IO = HID // 128       # 8 contraction blocks
JBLK = HID // 128     # 8 output-feature blocks
KT = T // 128         # 16 key tiles
TT = TL // 128        # 8 local row tiles
TB = TL // 512        # 2 local row blocks

REPLICA_GROUPS = [[0, 1], [2, 3], [4, 5], [6, 7]]


def _slot(h):
    return h % 4 + 4 * (h // 8)


def _half(h):
    return (h // 4) % 2


def _emit(tc, q_d, k_d, v_d, pm_d, wq_d, wk_d, wv_d, wo_d, out_d):
    nc = tc.nc
    with ExitStack() as ctx:
        const = ctx.enter_context(tc.tile_pool(name="const", bufs=1))
        ident = const.tile([128, 128], F32)
        make_identity(nc, ident)

        # pad mask -> additive bias, laid out [128(k%128), B, KT]
        pm_sb = const.tile([128, B, KT], I32)
        nc.sync.dma_start(pm_sb[:], pm_d.ap().rearrange("b (kt p) -> p b kt", p=128))
        pmf = const.tile([128, B, KT], F32)
        nc.vector.tensor_copy(pmf[:], pm_sb[:])
        biasT = const.tile([128, B, KT], F32)
        nc.vector.tensor_scalar_mul(biasT[:], pmf[:], NEG_INF)
        ones3 = const.tile([128, KT, 1], F32)
        nc.vector.memset(ones3[:], 1.0)

        qpT_pool = ctx.enter_context(tc.tile_pool(name="qpT", bufs=1))
        qpT = qpT_pool.tile([128, 8, TL], R32)   # [u*64+d, slot, q]
        ctxN_pool = ctx.enter_context(tc.tile_pool(name="ctxN", bufs=1))
        ctxN = ctxN_pool.tile([128, JBLK, TL], R32)  # [(h%2)*64+d, h//2, q]

        dram = ctx.enter_context(tc.tile_pool(name="dram", bufs=1, space="DRAM"))
        bounce = dram.tile([2 * TL, HID], F32)    # [kpT_local(j,t) ; vp_local(t,j)]
        gath = dram.tile([4 * TL, HID], F32)      # pair-gathered bounce

        # ---------------- phase A+B: projections ----------------
        with tc.tile_pool(name="w_pool", bufs=2) as wp, \
             tc.tile_pool(name="x_in", bufs=3) as xip, \
             tc.tile_pool(name="xT", bufs=2) as xtp, \
             tc.tile_pool(name="stage", bufs=4) as stp, \
             tc.tile_pool(name="qstage", bufs=3) as qsp, \
             tc.tile_pool(name="ps_t", bufs=2, space="PSUM") as pst, \
             tc.tile_pool(name="ps_p", bufs=3, space="PSUM") as psp:

            def load_w(w_d):
                # chunked per io-block so the first accumulation matmuls can
                # start before the whole 4MB weight arrives
                w = wp.tile([128, IO, HID], R32, tag="w")
                src = w_d.ap().rearrange("(io p) j -> p io j", p=128).bitcast(R32)
                for io in range(IO):
                    nc.sync.dma_start(w[:, io:io + 1, :], src[:, io:io + 1, :])
                return w

            def transpose_slab(x_d, tb):
                # x rows [tb*512, tb*512+512) transposed into [128(i), IO, 512(t)]
                slab = xtp.tile([128, IO, 512], R32, tag="slab")
                for tt4 in range(4):
                    tt = tb * 4 + tt4
                    x_sb = xip.tile([128, HID], F32, tag="x_in")
                    for ih2 in range(2):
                        nc.sync.dma_start(
                            x_sb[:, ih2 * 512:(ih2 + 1) * 512],
                            x_d.ap()[tt * 128:(tt + 1) * 128,
                                     ih2 * 512:(ih2 + 1) * 512])
                    for ih in range(2):
                        ps = pst.tile([128, 512], F32, tag="ps_t")
                        for i4 in range(4):
                            io = ih * 4 + i4
                            nc.tensor.matmul(
                                ps[:, i4 * 128:(i4 + 1) * 128],
                                x_sb[:, io * 128:(io + 1) * 128], ident[:],
                                is_transpose=True,
                                start=(i4 == 0), stop=(i4 == 3))
                        nc.vector.tensor_copy(
                            slab[:, ih * 4:(ih + 1) * 4, tt4 * 128:(tt4 + 1) * 128],
                            ps[:].rearrange("p (a b) -> p a b", a=4))
                return slab

            # k path: kpT_local [HID(j), TL(t)] -> bounce rows [0, TL)
            wk = load_w(wk_d)
            for tb in range(TB):
                slab = transpose_slab(k_d, tb)
                for jb in range(JBLK):
                    ps = psp.tile([128, 512], F32, tag="ps_p")
                    for io in range(IO):
                        nc.tensor.matmul(
                            ps[:], wk[:, io, jb * 128:(jb + 1) * 128], slab[:, io, :],
                            start=(io == 0), stop=(io == IO - 1))
                    st = stp.tile([128, 512], F32, tag="stage")
                    nc.vector.tensor_copy(st[:], ps[:])
                    nc.sync.dma_start(
                        bounce[jb * 128:(jb + 1) * 128, tb * 512:(tb + 1) * 512], st[:])

            # v path: vp_local [TL(t), HID(j)] -> bounce rows [TL, 2*TL)
            wv = load_w(wv_d)
            for tb in range(TB):
                slab = transpose_slab(v_d, tb)
                for tt4 in range(4):
                    tt = tb * 4 + tt4
                    for jh in range(2):
                        ps = psp.tile([128, 512], F32, tag="ps_p")
                        for io in range(IO):
                            nc.tensor.matmul(
                                ps[:], slab[:, io, tt4 * 128:(tt4 + 1) * 128],
                                wv[:, io, jh * 512:(jh + 1) * 512],
                                start=(io == 0), stop=(io == IO - 1))
                        st = stp.tile([128, 512], F32, tag="stage")
                        nc.vector.tensor_copy(st[:], ps[:])
                        nc.sync.dma_start(
                            bounce[TL + tt * 128:TL + (tt + 1) * 128,
                                   jh * 512:(jh + 1) * 512], st[:])

            # exchange k/v projections inside the core pair
            nc.gpsimd.collective_compute(
                "AllGather", mybir.AluOpType.bypass,
                replica_groups=REPLICA_GROUPS,
                ins=[bounce.opt()], outs=[gath.opt()])

            # q path -> resident qpT (overlaps the collective)
            wq = load_w(wq_d)
            for tb in range(TB):
                slab = transpose_slab(q_d, tb)
                for jb in range(JBLK):
                    ps = psp.tile([128, 512], F32, tag="ps_p")
                    for io in range(IO):
                        nc.tensor.matmul(
                            ps[:], wq[:, io, jb * 128:(jb + 1) * 128], slab[:, io, :],
                            start=(io == 0), stop=(io == IO - 1))
                    qs = qsp.tile([128, 512], R32, tag="qstage")
                    nc.vector.tensor_copy(qs[:], ps[:])
                    for r in range(2):
                        h = 2 * jb + r
                        s, u = _slot(h), _half(h)
                        nc.sync.dma_start(
                            qpT[u * 64:(u + 1) * 64, s, tb * 512:(tb + 1) * 512],
                            qs[r * 64:(r + 1) * 64, :])

        # ---------------- phase C: attention ----------------
        gath_r = gath[:].rearrange("(g rr) c -> g rr c", g=2)
        gath_v = gath[:].rearrange("(g half t8 p) c -> g half t8 p c",
                                   g=2, half=2, t8=8)
        # outlives phase C so phase D can consume it; the load overlaps attention
        wop = ctx.enter_context(tc.tile_pool(name="wo_pool", bufs=1))
        wo = wop.tile([128, JBLK, HID], R32)
        nc.sync.dma_start(
            wo[:], wo_d.ap().rearrange("(jo p) n -> p jo n", p=128).bitcast(R32))
        with tc.tile_pool(name="kpTs", bufs=3) as kps_p, \
             tc.tile_pool(name="vpm", bufs=6) as vpm_p, \
             tc.tile_pool(name="pt", bufs=4) as pt_p, \
             tc.tile_pool(name="norm", bufs=4) as nm_p, \
             tc.tile_pool(name="ctmp", bufs=2) as ct_p, \
             tc.tile_pool(name="ps_st", bufs=2, space="PSUM") as ps_st, \
             tc.tile_pool(name="ps_ctx", bufs=2, space="PSUM") as ps_ctx:
            for sp in range(8):
                hA = sp % 4 + 8 * (sp // 4)
                hB = hA + 4
                bb = sp % 4
                kpTs = kps_p.tile([128, T], R32, tag="kpTs")
                for u, h in ((0, hA), (1, hB)):
                    # split per gathered half so MM1 on the first 8 key tiles
                    # starts before the second half of the exchange lands
                    for g in range(2):
                        src = gath_r[g:g + 1, 64 * h:64 * h + 64, :].bitcast(R32)
                        nc.sync.dma_start(
                            kpTs[u * 64:(u + 1) * 64,
                                 g * TL:(g + 1) * TL],
                            src.rearrange("g1 p t -> p (g1 t)"))
                vpms = []
                for h in (hA, hB):
                    vpm = vpm_p.tile([128, KT, 65], R32, tag="vpm")
                    for g in range(2):
                        src = gath_v[g:g + 1, 1:2, :, :, 64 * h:64 * h + 64].bitcast(R32)
                        nc.sync.dma_start(
                            vpm[:, g * 8:(g + 1) * 8, 0:64],
                            src.rearrange("g1 one t8 p j -> p (g1 one t8) j"))
                    nc.vector.tensor_copy(vpm[:, :, 64:65], ones3[:])
                    vpms.append(vpm)
                vpmA, vpmB = vpms
                ctxA = ps_ctx.tile([65, TL], F32, tag="ctx")
                ctxB = ps_ctx.tile([65, TL], F32, tag="ctx")
                for kt in range(KT):
                    for qb in range(2):
                        # scores for both heads of the pair side by side:
                        # same bias row, one exp activation, 2-bank tile that
                        # double-buffers within the 8-bank PSUM budget
                        st = ps_st.tile([128, 1024], F32, tag="st")
                        for u in range(2):
                            nc.tensor.matmul(
                                st[:, u * 512:(u + 1) * 512],
                                kpTs[u * 64:(u + 1) * 64, kt * 128:(kt + 1) * 128],
                                qpT[u * 64:(u + 1) * 64, sp, qb * 512:(qb + 1) * 512],
                                start=True, stop=True,
                                tile_position=(u * 64, 0))
                        pt = pt_p.tile([128, 1024], R32, tag="pt")
                        nc.scalar.activation(pt[:], st[:], EXP,
                                             bias=biasT[:, bb, kt:kt + 1], scale=SCALE)
                        for u, (vpm, ctxZ) in enumerate(((vpmA, ctxA), (vpmB, ctxB))):
                            nc.tensor.matmul(
                                ctxZ[:, qb * 512:(qb + 1) * 512],
                                vpm[:, kt, :],
                                pt[:, u * 512:(u + 1) * 512],
                                start=(kt == 0), stop=(kt == KT - 1))
                # normalize: ctx[d, q] / denom[q]  (denom is matmul row 64)
                for h, ctxZ in ((hA, ctxA), (hB, ctxB)):
                    # NB: partition_broadcast reads physical partition 0 of its
                    # input tile (AP base is ignored), so rden must live at
                    # partition 0; DVE reciprocal handles the 64->0 shift.
                    rden = nm_p.tile([1, TL], F32, tag="rden")
                    nc.vector.reciprocal(rden[:], ctxZ[64:65, :])
                    rdenB = nm_p.tile([64, TL], F32, tag="rdenB")
                    nc.gpsimd.partition_broadcast(rdenB[:], rden[:])
                    jb, r = h // 2, h % 2
                    if r == 0:
                        nc.vector.tensor_mul(
                            ctxN[0:64, jb, :], ctxZ[0:64, :], rdenB[:])
                    else:
                        tmp = ct_p.tile([64, TL], R32, tag="ctmp")
                        nc.vector.tensor_mul(tmp[:], ctxZ[0:64, :], rdenB[:])
                        nc.sync.dma_start(ctxN[64:128, jb, :], tmp[:])

        # ---------------- phase D: output projection ----------------
        with tc.tile_pool(name="ostage", bufs=4) as osp, \
             tc.tile_pool(name="ps_o", bufs=4, space="PSUM") as pso:
            for tt in range(TT):
                for nb in range(2):
                    ps = pso.tile([128, 512], F32, tag="ps_o")
                    for jb in range(JBLK):
                        nc.tensor.matmul(
                            ps[:],
                            ctxN[:, jb, tt * 128:(tt + 1) * 128],
                            wo[:, jb, nb * 512:(nb + 1) * 512],
                            start=(jb == 0), stop=(jb == JBLK - 1))
                    ost = osp.tile([128, 512], F32, tag="ost")
                    nc.vector.tensor_copy(ost[:], ps[:])
                    nc.sync.dma_start(
                        out_d.ap()[tt * 128:(tt + 1) * 128,
                                   nb * 512:(nb + 1) * 512], ost[:])


def build():
    nc = bacc.Bacc("TRN2", target_bir_lowering=False, debug=False,
                   num_devices=N_CORES)
    q_d = nc.dram_tensor("q", [TL, HID], F32, kind="ExternalInput")
    k_d = nc.dram_tensor("k", [TL, HID], F32, kind="ExternalInput")
    v_d = nc.dram_tensor("v", [TL, HID], F32, kind="ExternalInput")
    pm_d = nc.dram_tensor("pad_mask", [B, T], I32, kind="ExternalInput")
    wq_d = nc.dram_tensor("Wq", [HID, HID], F32, kind="ExternalInput")
    wk_d = nc.dram_tensor("Wk", [HID, HID], F32, kind="ExternalInput")
    wv_d = nc.dram_tensor("Wv", [HID, HID], F32, kind="ExternalInput")
    wo_d = nc.dram_tensor("Wo", [HID, HID], F32, kind="ExternalInput")
    out_d = nc.dram_tensor("out", [TL, HID], F32, kind="ExternalOutput")

    with tile.TileContext(nc) as tc:
        _emit(tc, q_d, k_d, v_d, pm_d, wq_d, wk_d, wv_d, wo_d, out_d)
    nc.compile()
    return nc


_NC = None


def _get_nc():
    global _NC
    if _NC is None:
        _NC = build()
    return _NC


def kernel(**inputs):
    from concourse.bass_utils import run_bass_kernel_spmd

    q = np.ascontiguousarray(np.asarray(inputs["q"], dtype=np.float32))
    k = np.ascontiguousarray(np.asarray(inputs["k"], dtype=np.float32))
    v = np.ascontiguousarray(np.asarray(inputs["v"], dtype=np.float32))
    pm = np.ascontiguousarray(np.asarray(inputs["pad_mask"], dtype=np.int32))
    ws = {n: np.ascontiguousarray(np.asarray(inputs[n], dtype=np.float32))
          for n in ("Wq", "Wk", "Wv", "Wo")}

    in_maps = []
    for c in range(N_CORES):
        b, g = c // 2, c % 2
        sl = slice(g * TL, (g + 1) * TL)
        in_maps.append({
            "q": np.ascontiguousarray(q[b, sl]),
            "k": np.ascontiguousarray(k[b, sl]),
            "v": np.ascontiguousarray(v[b, sl]),
            "pad_mask": pm,
            **ws,
        })
    res = run_bass_kernel_spmd(_get_nc(), in_maps, list(range(N_CORES))).results
    out = np.empty((B, T, HID), np.float32)
    for c in range(N_CORES):
        b, g = c // 2, c % 2
        out[b, g * TL:(g + 1) * TL] = res[c]["out"]
    return out



# revision 14
# speedup vs baseline: 1.8381x; 1.8381x over previous
"""Trainium2 Bass kernel for multi-head attention (B=4, T=2048, HID=1024, H=16, D=64).

Sharding (8 NeuronCores): core c owns batch b = c//2 and query rows
g = c%2 (1024 of 2048). No collectives: both cores of a batch pair
redundantly project the batch's full key/value set, which is far cheaper
under this machine's interconnect than any inter-core exchange.

Mask semantics: the reference tiles the pad mask head-major
(jnp.tile(pad_mask, (H, 1)) against batch-major split heads), so head h
attends under mask row pad_mask[h % 4] for EVERY batch. The kernel takes
a per-class additive-bias input bias[4, T] (with a -3 shift folded in so
fp8 probabilities cannot overflow; softmax is shift-invariant).

Host staging (kernel() below): activations/weights pre-transposed into
PE-ready layouts and cast to bf16 (zero device-side input transposes),
Wq pre-scaled by D**-0.5, Wq/Wk/Wv pre-sliced per head-pair so weight
slices stream through a small rotating pool.

Device pipeline per core: per head-pair projections (kp -> [j,t],
vp -> [t,j] in fp8 with an appended ones column for the softmax
denominator, qp -> [j,t]) feed head-pipelined attention: scores st[k,q]
on PE, probabilities via one Scalar-engine exp per key tile (bias folds
the mask) written as fp8 (bf16 V keeps value precision; PE allows mixed
fp8 x bf16 operands), then PV with P chunks as the stationary
operand producing ctx[q, d+1] per query tile in its own PSUM bank
(kt-inner accumulation: matmul start=True clobbers bank-wide, so each
accumulator owns a bank and sees exactly one start). PV lags scores by
one head so exp latency hides. Per-partition softmax normalize on DVE.
A tail PE-transpose of ctx feeds the row-parallel output projection,
which tiles the full output with no reduction.
"""

from contextlib import ExitStack

import numpy as np

import concourse.bacc as bacc
import concourse.mybir as mybir
import concourse.tile as tile
from concourse.masks import make_identity

F32 = mybir.dt.float32
BF16 = mybir.dt.bfloat16
FP8 = mybir.dt.float8e4
EXP = mybir.ActivationFunctionType.Exp

B, T, HID, H, D = 4, 2048, 1024, 16, 64
TQ = T // 2            # query rows owned by one core
KT = T // 128          # 16 key tiles
QT = TQ // 128         # 8 query tiles
IO = HID // 128        # 8 contraction blocks
HP = H // 2            # 8 head pairs
NCLS = 4               # pad-mask classes (head h uses class h % 4)
N_CORES = 8
NEG_INF = -1.0e9
BIAS_SHIFT = -3.0      # keeps exp() outputs well inside fp8e4m3 range


def _emit(tc, qT_d, kT_d, vT_d, bias_d, wqs_d, wks_d, wvs_d, wo_d, out_d):
    nc = tc.nc
    with ExitStack() as ctx:
        const = ctx.enter_context(tc.tile_pool(name="const", bufs=1))
        ident = const.tile([128, 128], BF16)
        make_identity(nc, ident)
        bias_sb = const.tile([128, NCLS, KT], F32)
        nc.sync.dma_start(
            bias_sb[:], bias_d.ap().rearrange("c (kt p) -> p c kt", p=128))
        ctxN = const.tile([128, QT, HID], BF16)   # [q%128, qt, j] normalized ctx

        big = ctx.enter_context(
            tc.tile_pool(name="big", bufs=3, space="PSUM"))

        with tc.tile_pool(name="wsl", bufs=3) as wsp, \
             tc.tile_pool(name="xin", bufs=1) as xip, \
             tc.tile_pool(name="kpp", bufs=3) as kpp, \
             tc.tile_pool(name="qpp", bufs=3) as qpp, \
             tc.tile_pool(name="vpp", bufs=3) as vpp, \
             tc.tile_pool(name="pt", bufs=34) as ptp, \
             tc.tile_pool(name="rden", bufs=4) as rdp, \
             tc.tile_pool(name="ctxps", bufs=2, space="PSUM") as cxp:

            def dma_w(hp):
                # per-head-pair weight slices, host-staged contiguous
                wk = wsp.tile([128, IO, 128], BF16, tag="wk")
                nc.sync.dma_start(wk[:], wks_d.ap()[hp])
                wq = wsp.tile([128, IO, 128], BF16, tag="wq")
                nc.sync.dma_start(wq[:], wqs_d.ap()[hp])
                wv = wsp.tile([128, IO, 128], BF16, tag="wv")
                nc.sync.dma_start(wv[:], wvs_d.ap()[hp])
                return wk, wq, wv

            # ---- input DMAs: first weight slices right after kT so the
            # first projection isn't stuck behind the bulk q/v loads ----
            kT_sb = xip.tile([128, IO, T], BF16, tag="kT")
            nc.sync.dma_start(kT_sb[:], kT_d.ap().rearrange("(io p) t -> p io t", p=128))
            ws = {0: dma_w(0)}
            qT_sb = xip.tile([128, IO, TQ], BF16, tag="qT")
            nc.sync.dma_start(qT_sb[:], qT_d.ap().rearrange("(io p) t -> p io t", p=128))
            ws[1] = dma_w(1)
            vT_sb = xip.tile([128, IO, T], BF16, tag="vT")
            nc.sync.dma_start(vT_sb[:], vT_d.ap().rearrange("(io p) t -> p io t", p=128))
            ws[2] = dma_w(2)

            def make_proj(hp, wk, wq, wv):
                """Allocate the pair's projection tiles; return (tiles, gen).

                The generator emits the projection matmuls in ~0.5-1.7us
                chunks so the driver can interleave them between score
                tiles, keeping PE fed while the Scalar engine drains exps.
                """
                kpTt = kpp.tile([128, T], BF16, tag="kpT")
                qpTt = qpp.tile([128, TQ], BF16, tag="qpT")
                vpmt = vpp.tile([128, KT, 2, 65], BF16, tag="vpm")
                nc.gpsimd.memset(vpmt[:, :, :, 64:65], 1.0)

                def gen():
                    for tg in range(2):
                        ps = big.tile([128, 1024], F32, tag="big")
                        for half in range(2):
                            for io in range(IO):
                                nc.tensor.matmul(
                                    ps[:, half * 512:(half + 1) * 512],
                                    wk[:, io, :],
                                    kT_sb[:, io, tg * 1024 + half * 512:
                                          tg * 1024 + (half + 1) * 512],
                                    start=(io == 0), stop=(io == IO - 1))
                            if half == 1:
                                nc.vector.tensor_copy(
                                    kpTt[:, tg * 1024:(tg + 1) * 1024], ps[:])
                            yield
                    ps = big.tile([128, 1024], F32, tag="big")
                    for half in range(2):
                        for io in range(IO):
                            nc.tensor.matmul(
                                ps[:, half * 512:(half + 1) * 512],
                                wq[:, io, :],
                                qT_sb[:, io, half * 512:(half + 1) * 512],
                                start=(io == 0), stop=(io == IO - 1))
                        if half == 1:
                            nc.vector.tensor_copy(qpTt[:], ps[:])
                        yield
                    for tg in range(2):
                        ps = big.tile([128, 1024], F32, tag="big")
                        for tt8 in range(8):
                            tt = tg * 8 + tt8
                            for io in range(IO):
                                nc.tensor.matmul(
                                    ps[:, tt8 * 128:(tt8 + 1) * 128],
                                    vT_sb[:, io, tt * 128:(tt + 1) * 128],
                                    wv[:, io, :],
                                    start=(io == 0), stop=(io == IO - 1))
                            if tt8 == 7:
                                nc.vector.tensor_copy(
                                    vpmt[:, tg * 8:(tg + 1) * 8, :, 0:64],
                                    ps[:].rearrange("p (tt hh d) -> p tt hh d",
                                                    tt=8, hh=2))
                            if tt8 % 2 == 1:
                                yield

                return (kpTt, qpTt, vpmt), gen()

            N_PROJ_PULLS = 14  # yields per proj generator (4 kp + 2 qp + 8 vp)

            def attn_tick(h, kpTt, qpTt, kt):
                # one score tile + its exp; returns the resident P tile
                r, c = h % 2, h % NCLS
                st = big.tile([128, 1024], F32, tag="big")
                for half in range(2):
                    nc.tensor.matmul(
                        st[:, half * 512:(half + 1) * 512],
                        kpTt[r * 64:(r + 1) * 64, kt * 128:(kt + 1) * 128],
                        qpTt[r * 64:(r + 1) * 64, half * 512:(half + 1) * 512],
                        start=True, stop=True,
                        tile_position=(r * 64, 0))
                pt = ptp.tile([128, 1024], FP8, tag="pt", bufs=34)
                nc.scalar.activation(pt[:], st[:], EXP,
                                     bias=bias_sb[:, c, kt:kt + 1])
                return pt

            def pv_gen(h, pts, vpmt):
                # kt-inner PV: each qt accumulator owns one PSUM bank, so it
                # sees exactly one start=True (start clobbers bank-wide)
                r = h % 2
                for qt in range(QT):
                    cx = cxp.tile([128, 128], F32, tag="cx")
                    for kt in range(KT):
                        nc.tensor.matmul(
                            cx[:, 0:65],
                            pts[kt][:, qt * 128:(qt + 1) * 128],
                            vpmt[:, kt, r, :],
                            start=(kt == 0), stop=(kt == KT - 1))
                    rden = rdp.tile([128, 1], F32, tag="rden")
                    nc.vector.reciprocal(rden[:], cx[:, 64:65])
                    nc.vector.tensor_scalar_mul(
                        ctxN[:, qt, h * 64:(h + 1) * 64],
                        cx[:, 0:64], rden[:])
                    yield

            # ---- pipelined emission: weight DMAs 2 stages ahead, projection
            # chunks interleaved between score tiles one stage ahead, PV one
            # head behind scores (also interleaved) ----
            tiles0, pg0 = make_proj(0, *ws[0])
            for _ in pg0:
                pass
            projs = {0: tiles0}
            pend = None
            pg = None
            for hp in range(HP):
                if hp + 3 < HP:
                    ws[hp + 3] = dma_w(hp + 3)
                if hp + 1 < HP:
                    projs[hp + 1], pg = make_proj(hp + 1, *ws[hp + 1])
                    ws.pop(hp + 1)
                else:
                    pg = None
                kpTt, qpTt, vpmt = projs.pop(hp)
                pulled = 0
                pvg = None
                for r in range(2):
                    h = 2 * hp + r
                    pvg = pv_gen(*pend) if pend is not None else None
                    pts = []
                    for kt in range(KT):
                        pts.append(attn_tick(h, kpTt, qpTt, kt))
                        if pvg is not None and kt % 2 == 1:
                            next(pvg, None)
                        tick = r * KT + kt + 1
                        want = (tick * N_PROJ_PULLS) // (2 * KT)
                        while pg is not None and pulled < want:
                            if next(pg, StopIteration) is StopIteration:
                                pg = None
                                break
                            pulled += 1
                    pend = (h, pts, vpmt)
            for _ in pv_gen(*pend):
                pass

        # ---------------- tail: transpose ctx + output projection ----------
        with tc.tile_pool(name="ctxT", bufs=1) as ctp, \
             tc.tile_pool(name="ostage", bufs=2) as osp, \
             tc.tile_pool(name="tailps", bufs=2, space="PSUM") as tlp:
            # Wo loads into SBUF freed by the projection pools; the DMA can
            # start as soon as the last projection read retires
            wo_sb = ctp.tile([128, IO, HID], BF16, tag="wo")
            nc.sync.dma_start(wo_sb[:], wo_d.ap().rearrange("(io p) j -> p io j", p=128))
            ctxT = ctp.tile([128, IO, TQ], BF16)
            for jb in range(IO):
                ps = tlp.tile([128, 1024], BF16, tag="tpose")
                for qt in range(QT):
                    nc.tensor.matmul(
                        ps[:, qt * 128:(qt + 1) * 128],
                        ctxN[:, qt, jb * 128:(jb + 1) * 128], ident[:],
                        is_transpose=True, start=True, stop=True)
                nc.vector.tensor_copy(ctxT[:, jb, :], ps[:])
            for tt in range(QT):
                pso = big.tile([128, 1024], F32, tag="big")
                for half in range(2):
                    for jb in range(IO):
                        nc.tensor.matmul(
                            pso[:, half * 512:(half + 1) * 512],
                            ctxT[:, jb, tt * 128:(tt + 1) * 128],
                            wo_sb[:, jb, half * 512:(half + 1) * 512],
                            start=(jb == 0), stop=(jb == IO - 1))
                ost = osp.tile([128, 1024], F32, tag="ost")
                nc.vector.tensor_copy(ost[:], pso[:])
                nc.sync.dma_start(out_d.ap()[tt * 128:(tt + 1) * 128, :], ost[:])


def build():
    nc = bacc.Bacc("TRN2", target_bir_lowering=False, debug=False,
                   num_devices=N_CORES)
    qT_d = nc.dram_tensor("qT", [HID, TQ], BF16, kind="ExternalInput")
    kT_d = nc.dram_tensor("kT", [HID, T], BF16, kind="ExternalInput")
    vT_d = nc.dram_tensor("vT", [HID, T], BF16, kind="ExternalInput")
    bias_d = nc.dram_tensor("bias", [NCLS, T], F32, kind="ExternalInput")
    wqs_d = nc.dram_tensor("Wqs", [HP, 128, IO, 128], BF16, kind="ExternalInput")
    wks_d = nc.dram_tensor("Wks", [HP, 128, IO, 128], BF16, kind="ExternalInput")
    wvs_d = nc.dram_tensor("Wvs", [HP, 128, IO, 128], BF16, kind="ExternalInput")
    wo_d = nc.dram_tensor("Wo", [HID, HID], BF16, kind="ExternalInput")
    out_d = nc.dram_tensor("out", [TQ, HID], F32, kind="ExternalOutput")

    with tile.TileContext(nc) as tc:
        _emit(tc, qT_d, kT_d, vT_d, bias_d, wqs_d, wks_d, wvs_d, wo_d, out_d)
    nc.compile()
    return nc


_NC = None


def _get_nc():
    global _NC
    if _NC is None:
        _NC = build()
    return _NC


def _slice_weight(w):
    # [HID, HID] -> [HP, 128, IO, 128]: staged[hp, p, io, jj] = w[io*128+p, hp*128+jj]
    return np.ascontiguousarray(
        w.reshape(IO, 128, HP, 128).transpose(2, 1, 0, 3))


def kernel(**inputs):
    import ml_dtypes
    from concourse.bass_utils import run_bass_kernel_spmd

    bf16 = ml_dtypes.bfloat16
    q = np.asarray(inputs["q"], dtype=np.float32)
    k = np.asarray(inputs["k"], dtype=np.float32)
    v = np.asarray(inputs["v"], dtype=np.float32)
    pm = np.asarray(inputs["pad_mask"], dtype=np.float32)
    wqs = _slice_weight(
        (np.asarray(inputs["Wq"], dtype=np.float32) * (D ** -0.5)).astype(bf16))
    wks = _slice_weight(np.asarray(inputs["Wk"], dtype=np.float32).astype(bf16))
    wvs = _slice_weight(np.asarray(inputs["Wv"], dtype=np.float32).astype(bf16))
    wo = np.asarray(inputs["Wo"], dtype=np.float32).astype(bf16)

    # head h is masked by pad_mask[h % 4] (reference tiles the mask
    # head-major); shift keeps exp() inside fp8e4m3 range
    bias = (pm[0:NCLS] * NEG_INF + BIAS_SHIFT).astype(np.float32)

    kTs = [np.ascontiguousarray(k[b].T.astype(bf16)) for b in range(B)]
    vTs = [np.ascontiguousarray(v[b].T.astype(bf16)) for b in range(B)]

    in_maps = []
    for c in range(N_CORES):
        b, g = c // 2, c % 2
        qT = np.ascontiguousarray(q[b, g * TQ:(g + 1) * TQ, :].T.astype(bf16))
        in_maps.append({
            "qT": qT,
            "kT": kTs[b],
            "vT": vTs[b],
            "bias": bias,
            "Wqs": wqs, "Wks": wks, "Wvs": wvs, "Wo": wo,
        })
    res = run_bass_kernel_spmd(_get_nc(), in_maps, list(range(N_CORES))).results
    out = np.empty((B, T, HID), np.float32)
    for c in range(N_CORES):
        b, g = c // 2, c % 2
        out[b, g * TQ:(g + 1) * TQ] = res[c]["out"]
    return out


# revision 15
# speedup vs baseline: 1.8522x; 1.0077x over previous
"""Trainium2 Bass kernel for multi-head attention (B=4, T=2048, HID=1024, H=16, D=64).

Sharding (8 NeuronCores): core c owns batch b = c//2 and query rows
g = c%2 (1024 of 2048). No collectives: both cores of a batch pair
redundantly project the batch's full key/value set, which is far cheaper
under this machine's interconnect than any inter-core exchange.

Mask semantics: the reference tiles the pad mask head-major
(jnp.tile(pad_mask, (H, 1)) against batch-major split heads), so head h
attends under mask row pad_mask[h % 4] for EVERY batch. The kernel takes
a per-class additive-bias input bias[4, T] (with a -3 shift folded in so
fp8 probabilities cannot overflow; softmax is shift-invariant).

Host staging (kernel() below): activations/weights pre-transposed into
PE-ready layouts and cast to bf16 (zero device-side input transposes),
Wq pre-scaled by D**-0.5, Wq/Wk/Wv pre-sliced per head-pair so weight
slices stream through a small rotating pool.

Device pipeline per core: per head-pair projections (kp -> [j,t],
vp -> [t,j] in fp8 with an appended ones column for the softmax
denominator, qp -> [j,t]) feed head-pipelined attention: scores st[k,q]
on PE, probabilities via one Scalar-engine exp per key tile (bias folds
the mask), then PV with P chunks as the stationary operand producing ctx[q, d+1] per query tile in its own PSUM bank
(kt-inner accumulation: matmul start=True clobbers bank-wide, so each
accumulator owns a bank and sees exactly one start). PV lags scores by
one head so exp latency hides. Per-partition softmax normalize on DVE.
A tail PE-transpose of ctx feeds the row-parallel output projection,
which tiles the full output with no reduction.
"""

from contextlib import ExitStack

import numpy as np

import concourse.bacc as bacc
import concourse.mybir as mybir
import concourse.tile as tile
from concourse.masks import make_identity

F32 = mybir.dt.float32
BF16 = mybir.dt.bfloat16
FP8 = mybir.dt.float8e4
EXP = mybir.ActivationFunctionType.Exp

B, T, HID, H, D = 4, 2048, 1024, 16, 64
TQ = T // 2            # query rows owned by one core
KT = T // 128          # 16 key tiles
QT = TQ // 128         # 8 query tiles
IO = HID // 128        # 8 contraction blocks
HP = H // 2            # 8 head pairs
NCLS = 4               # pad-mask classes (head h uses class h % 4)
N_CORES = 8
NEG_INF = -1.0e9
BIAS_SHIFT = -3.0      # keeps exp() outputs well inside fp8e4m3 range


def _emit(tc, qT_d, kT_d, vT_d, bias_d, wqs_d, wks_d, wvs_d, wo_d, out_d):
    nc = tc.nc
    with ExitStack() as ctx:
        const = ctx.enter_context(tc.tile_pool(name="const", bufs=1))
        ident = const.tile([128, 128], BF16)
        make_identity(nc, ident)
        bias_sb = const.tile([128, NCLS, KT], F32)
        nc.sync.dma_start(
            bias_sb[:], bias_d.ap().rearrange("c (kt p) -> p c kt", p=128))
        ctxN = const.tile([128, QT, HID], BF16)   # [q%128, qt, j] normalized ctx

        big = ctx.enter_context(
            tc.tile_pool(name="big", bufs=3, space="PSUM"))

        with tc.tile_pool(name="wsl", bufs=2) as wsp, \
             tc.tile_pool(name="xin", bufs=1) as xip, \
             tc.tile_pool(name="kpp", bufs=2) as kpp, \
             tc.tile_pool(name="qpp", bufs=2) as qpp, \
             tc.tile_pool(name="vpp", bufs=3) as vpp, \
             tc.tile_pool(name="pt", bufs=34) as ptp, \
             tc.tile_pool(name="rden", bufs=4) as rdp, \
             tc.tile_pool(name="ctxps", bufs=2, space="PSUM") as cxp:

            def dma_w(hp):
                # per-head-pair weight slices, host-staged contiguous
                wk = wsp.tile([128, IO, 128], BF16, tag="wk")
                nc.sync.dma_start(wk[:], wks_d.ap()[hp])
                wq = wsp.tile([128, IO, 128], BF16, tag="wq")
                nc.sync.dma_start(wq[:], wqs_d.ap()[hp])
                wv = wsp.tile([128, IO, 128], BF16, tag="wv")
                nc.sync.dma_start(wv[:], wvs_d.ap()[hp])
                return wk, wq, wv

            # ---- input DMAs: first weight slices and the first kT half
            # lead, so the first projection starts ~7us in ----
            ws = {0: dma_w(0)}
            kT_sb = xip.tile([128, IO, T], BF16, tag="kT")
            kT_src = kT_d.ap().rearrange("(io p) t -> p io t", p=128)
            nc.sync.dma_start(kT_sb[:, :, 0:1024], kT_src[:, :, 0:1024])
            qT_sb = xip.tile([128, IO, TQ], BF16, tag="qT")
            nc.sync.dma_start(qT_sb[:], qT_d.ap().rearrange("(io p) t -> p io t", p=128))
            nc.sync.dma_start(kT_sb[:, :, 1024:T], kT_src[:, :, 1024:T])
            ws[1] = dma_w(1)
            vT_sb = xip.tile([128, IO, T], BF16, tag="vT")
            vT_src = vT_d.ap().rearrange("(io p) t -> p io t", p=128)
            nc.sync.dma_start(vT_sb[:, :, 0:1024], vT_src[:, :, 0:1024])
            ws[2] = dma_w(2)
            nc.sync.dma_start(vT_sb[:, :, 1024:T], vT_src[:, :, 1024:T])

            def make_proj(hp, wk, wq, wv):
                """Allocate the pair's projection tiles; return (tiles, gen).

                The generator emits the projection matmuls in ~0.5-1.7us
                chunks so the driver can interleave them between score
                tiles, keeping PE fed while the Scalar engine drains exps.
                """
                kpTt = kpp.tile([128, T], BF16, tag="kpT")
                qpTt = qpp.tile([128, TQ], BF16, tag="qpT")
                vpmt = vpp.tile([128, KT, 2, 65], BF16, tag="vpm")
                nc.gpsimd.memset(vpmt[:, :, :, 64:65], 1.0)

                def gen():
                    for tg in range(2):
                        ps = big.tile([128, 1024], F32, tag="big")
                        for half in range(2):
                            for io in range(IO):
                                nc.tensor.matmul(
                                    ps[:, half * 512:(half + 1) * 512],
                                    wk[:, io, :],
                                    kT_sb[:, io, tg * 1024 + half * 512:
                                          tg * 1024 + (half + 1) * 512],
                                    start=(io == 0), stop=(io == IO - 1))
                            if half == 1:
                                nc.vector.tensor_copy(
                                    kpTt[:, tg * 1024:(tg + 1) * 1024], ps[:])
                            yield
                    ps = big.tile([128, 1024], F32, tag="big")
                    for half in range(2):
                        for io in range(IO):
                            nc.tensor.matmul(
                                ps[:, half * 512:(half + 1) * 512],
                                wq[:, io, :],
                                qT_sb[:, io, half * 512:(half + 1) * 512],
                                start=(io == 0), stop=(io == IO - 1))
                        if half == 1:
                            nc.vector.tensor_copy(qpTt[:], ps[:])
                        yield
                    for tg in range(2):
                        ps = big.tile([128, 1024], F32, tag="big")
                        for tt8 in range(8):
                            tt = tg * 8 + tt8
                            for io in range(IO):
                                nc.tensor.matmul(
                                    ps[:, tt8 * 128:(tt8 + 1) * 128],
                                    vT_sb[:, io, tt * 128:(tt + 1) * 128],
                                    wv[:, io, :],
                                    start=(io == 0), stop=(io == IO - 1))
                            if tt8 == 7:
                                nc.vector.tensor_copy(
                                    vpmt[:, tg * 8:(tg + 1) * 8, :, 0:64],
                                    ps[:].rearrange("p (tt hh d) -> p tt hh d",
                                                    tt=8, hh=2))
                            if tt8 % 2 == 1:
                                yield

                return (kpTt, qpTt, vpmt), gen()

            N_PROJ_PULLS = 14  # yields per proj generator (4 kp + 2 qp + 8 vp)

            def attn_tick(h, kpTt, qpTt, kt):
                # one score tile + its exp; returns the resident P tile
                r, c = h % 2, h % NCLS
                st = big.tile([128, 1024], F32, tag="big")
                for half in range(2):
                    nc.tensor.matmul(
                        st[:, half * 512:(half + 1) * 512],
                        kpTt[r * 64:(r + 1) * 64, kt * 128:(kt + 1) * 128],
                        qpTt[r * 64:(r + 1) * 64, half * 512:(half + 1) * 512],
                        start=True, stop=True,
                        tile_position=(r * 64, 0))
                pt = ptp.tile([128, 1024], BF16, tag="pt", bufs=34)
                nc.scalar.activation(pt[:], st[:], EXP,
                                     bias=bias_sb[:, c, kt:kt + 1])
                return pt

            def pv_gen(h, pts, vpmt):
                # kt-inner PV: each qt accumulator owns one PSUM bank, so it
                # sees exactly one start=True (start clobbers bank-wide)
                r = h % 2
                for qt in range(QT):
                    cx = cxp.tile([128, 128], F32, tag="cx")
                    for kt in range(KT):
                        nc.tensor.matmul(
                            cx[:, 0:65],
                            pts[kt][:, qt * 128:(qt + 1) * 128],
                            vpmt[:, kt, r, :],
                            start=(kt == 0), stop=(kt == KT - 1))
                    rden = rdp.tile([128, 1], F32, tag="rden")
                    nc.vector.reciprocal(rden[:], cx[:, 64:65])
                    nc.vector.tensor_scalar_mul(
                        ctxN[:, qt, h * 64:(h + 1) * 64],
                        cx[:, 0:64], rden[:])
                    yield

            # ---- pipelined emission: weight DMAs 2 stages ahead, projection
            # chunks interleaved between score tiles one stage ahead, PV one
            # head behind scores (also interleaved) ----
            tiles0, pg0 = make_proj(0, *ws[0])
            for _ in pg0:
                pass
            projs = {0: tiles0}
            pend = None
            pg = None
            for hp in range(HP):
                if hp + 2 < HP and hp + 2 not in ws and hp + 2 > 2:
                    ws[hp + 2] = dma_w(hp + 2)
                if hp + 1 < HP:
                    projs[hp + 1], pg = make_proj(hp + 1, *ws[hp + 1])
                    ws.pop(hp + 1)
                else:
                    pg = None
                kpTt, qpTt, vpmt = projs.pop(hp)
                pulled = 0
                pvg = None
                for r in range(2):
                    h = 2 * hp + r
                    pvg = pv_gen(*pend) if pend is not None else None
                    pts = []
                    for kt in range(KT):
                        pts.append(attn_tick(h, kpTt, qpTt, kt))
                        if pvg is not None and kt % 2 == 1:
                            next(pvg, None)
                        tick = r * KT + kt + 1
                        want = (tick * N_PROJ_PULLS) // (2 * KT)
                        while pg is not None and pulled < want:
                            if next(pg, StopIteration) is StopIteration:
                                pg = None
                                break
                            pulled += 1
                    pend = (h, pts, vpmt)
            for _ in pv_gen(*pend):
                pass

        # ---------------- tail: transpose ctx + output projection ----------
        with tc.tile_pool(name="ctxT", bufs=1) as ctp, \
             tc.tile_pool(name="ostage", bufs=2) as osp, \
             tc.tile_pool(name="tailps", bufs=2, space="PSUM") as tlp:
            # Wo loads into SBUF freed by the projection pools; the DMA can
            # start as soon as the last projection read retires
            wo_sb = ctp.tile([128, IO, HID], BF16, tag="wo")
            nc.sync.dma_start(wo_sb[:], wo_d.ap().rearrange("(io p) j -> p io j", p=128))
            ctxT = ctp.tile([128, IO, TQ], BF16)
            for jb in range(IO):
                ps = tlp.tile([128, 1024], BF16, tag="tpose")
                for qt in range(QT):
                    nc.tensor.matmul(
                        ps[:, qt * 128:(qt + 1) * 128],
                        ctxN[:, qt, jb * 128:(jb + 1) * 128], ident[:],
                        is_transpose=True, start=True, stop=True)
                nc.vector.tensor_copy(ctxT[:, jb, :], ps[:])
            for tt in range(QT):
                pso = big.tile([128, 1024], F32, tag="big")
                for half in range(2):
                    for jb in range(IO):
                        nc.tensor.matmul(
                            pso[:, half * 512:(half + 1) * 512],
                            ctxT[:, jb, tt * 128:(tt + 1) * 128],
                            wo_sb[:, jb, half * 512:(half + 1) * 512],
                            start=(jb == 0), stop=(jb == IO - 1))
                ost = osp.tile([128, 1024], F32, tag="ost")
                nc.vector.tensor_copy(ost[:], pso[:])
                nc.sync.dma_start(out_d.ap()[tt * 128:(tt + 1) * 128, :], ost[:])


def build():
    nc = bacc.Bacc("TRN2", target_bir_lowering=False, debug=False,
                   num_devices=N_CORES)
    qT_d = nc.dram_tensor("qT", [HID, TQ], BF16, kind="ExternalInput")
    kT_d = nc.dram_tensor("kT", [HID, T], BF16, kind="ExternalInput")
    vT_d = nc.dram_tensor("vT", [HID, T], BF16, kind="ExternalInput")
    bias_d = nc.dram_tensor("bias", [NCLS, T], F32, kind="ExternalInput")
    wqs_d = nc.dram_tensor("Wqs", [HP, 128, IO, 128], BF16, kind="ExternalInput")
    wks_d = nc.dram_tensor("Wks", [HP, 128, IO, 128], BF16, kind="ExternalInput")
    wvs_d = nc.dram_tensor("Wvs", [HP, 128, IO, 128], BF16, kind="ExternalInput")
    wo_d = nc.dram_tensor("Wo", [HID, HID], BF16, kind="ExternalInput")
    out_d = nc.dram_tensor("out", [TQ, HID], F32, kind="ExternalOutput")

    with tile.TileContext(nc) as tc:
        _emit(tc, qT_d, kT_d, vT_d, bias_d, wqs_d, wks_d, wvs_d, wo_d, out_d)
    nc.compile()
    return nc


_NC = None


def _get_nc():
    global _NC
    if _NC is None:
        _NC = build()
    return _NC


def _slice_weight(w):
    # [HID, HID] -> [HP, 128, IO, 128]: staged[hp, p, io, jj] = w[io*128+p, hp*128+jj]
    return np.ascontiguousarray(
        w.reshape(IO, 128, HP, 128).transpose(2, 1, 0, 3))


def kernel(**inputs):
    import ml_dtypes
    from concourse.bass_utils import run_bass_kernel_spmd

    bf16 = ml_dtypes.bfloat16
    q = np.asarray(inputs["q"], dtype=np.float32)
    k = np.asarray(inputs["k"], dtype=np.float32)
    v = np.asarray(inputs["v"], dtype=np.float32)
    pm = np.asarray(inputs["pad_mask"], dtype=np.float32)
    wqs = _slice_weight(
        (np.asarray(inputs["Wq"], dtype=np.float32) * (D ** -0.5)).astype(bf16))
    wks = _slice_weight(np.asarray(inputs["Wk"], dtype=np.float32).astype(bf16))
    wvs = _slice_weight(np.asarray(inputs["Wv"], dtype=np.float32).astype(bf16))
    wo = np.asarray(inputs["Wo"], dtype=np.float32).astype(bf16)

    # head h is masked by pad_mask[h % 4] (reference tiles the mask
    # head-major); shift keeps exp() inside fp8e4m3 range
    bias = (pm[0:NCLS] * NEG_INF + BIAS_SHIFT).astype(np.float32)

    kTs = [np.ascontiguousarray(k[b].T.astype(bf16)) for b in range(B)]
    vTs = [np.ascontiguousarray(v[b].T.astype(bf16)) for b in range(B)]

    in_maps = []
    for c in range(N_CORES):
        b, g = c // 2, c % 2
        qT = np.ascontiguousarray(q[b, g * TQ:(g + 1) * TQ, :].T.astype(bf16))
        in_maps.append({
            "qT": qT,
            "kT": kTs[b],
            "vT": vTs[b],
            "bias": bias,
            "Wqs": wqs, "Wks": wks, "Wvs": wvs, "Wo": wo,
        })
    res = run_bass_kernel_spmd(_get_nc(), in_maps, list(range(N_CORES))).results
    out = np.empty((B, T, HID), np.float32)
    for c in range(N_CORES):
        b, g = c // 2, c % 2
        out[b, g * TQ:(g + 1) * TQ] = res[c]["out"]
    return out
